# revision 2
# baseline (speedup 1.0000x reference)
"""Bass/Tile Trainium2 kernel for nn_Generator_LSTM_23433341567859.

Pipeline (per reference.py):
  1. LSTM0 (tanh) over noise [B=65536, T=32, F=16], D=8 -> final (h, c)
  2. BatchNorm(h), BatchNorm(c) (training-mode, stats over full batch)
  3. 14 autoregressive LSTM1 (relu) steps, input = h itself -> hs [14, B, 8]
  4. bars = reverse(hs), BatchNorm over (B, 14) per channel
  5. LSTM2 (relu, return_sequences) over bars -> out [B, 14, 4]

Sharding: pure data-parallel over batch across 8 NeuronCores; BN stats
all-reduced with a tiny [8,4]/[8,2] f32 AllReduce.

Device layout (per core, B_loc = 8192):
  batch index b = 2048*g + 512*a + n   (g: 0..3, a: 0..3, n: 0..511)
  All per-element tensors are [128, 512] SBUF/PSUM tiles with
  partition p = 32*g + 8*a + d  (d = channel 0..7), free = n.
  One PSUM tensor per gate (i, f, o, g~); each gate's pre-activation is
  produced by ONE K=128 matmul with a 16-tile block-diagonal stationary
  (plus two K=128 matmuls for the LSTM0 input term), so every matmul and
  every elementwise op runs with all 128 partitions busy.
"""

import os
import numpy as np
import ml_dtypes

BF16 = ml_dtypes.bfloat16

B, T, F, D, Do = 65536, 32, 16, 8, 4
NCORES = 8
BL = B // NCORES          # 8192 batch rows per core
ROWS = 14
EPS = 1e-3
GATES = ("i", "f", "o", "g")      # our processing order (g last)
KCOL = {"i": 0, "f": 8, "g": 16, "o": 24}     # keras i,f,g,o col offsets (x8)
KCOL2 = {"i": 0, "f": 4, "g": 8, "o": 12}     # LSTM2 (Do=4) col offsets


# --------------------------------------------------------------------------
# Host-side packing
# --------------------------------------------------------------------------

def _pack_x(noise):
    """noise [B, T, F] f32 -> per-core xt [NCORES, T, 128, 1024]:
    xt[c, t, 64*(g%2) + 16a + f, 512*(g//2) + n]
        = noise[c*8192 + 2048g + 512a + n, t, f]"""
    x = np.asarray(noise, np.float32).reshape(NCORES, 2, 2, 4, 512, T, F)
    #                                   [c, gh, gl, a, n, t, f]
    x = x.transpose(0, 5, 2, 3, 6, 1, 4)       # [c, t, gl, a, f, gh, n]
    return np.ascontiguousarray(x).reshape(NCORES, T, 128, 1024).astype(BF16)


def _s_x0(W0, G):
    """LSTM0 input stationary [128, 64]:
    S[64*gl + 16a + f, 32*gl + 8a + d] = W0[f, kcol(G)+d]  (gl = 0, 1)."""
    W0 = np.asarray(W0, np.float32)
    kc = KCOL[G]
    S = np.zeros((128, 64), np.float32)
    for gl in range(2):
        for a in range(4):
            S[64 * gl + 16 * a:64 * gl + 16 * a + 16,
              32 * gl + 8 * a:32 * gl + 8 * a + 8] = W0[:, kc:kc + 8]
    return S


def _s_h(U, G, din, dout):
    """Recurrent stationary [128, 128], 16-tile block-diagonal:
    S[32g + 8a + r, 32g + 8a + d] = U[r, kcol+d]  (r < din, d < dout)."""
    U = np.asarray(U, np.float32)
    kc = (KCOL if dout == 8 else KCOL2)[G]
    S = np.zeros((128, 128), np.float32)
    for g in range(4):
        for a in range(4):
            r0 = 32 * g + 8 * a
            S[r0:r0 + din, 32 * g + (8 if dout == 8 else 4) * a:
              32 * g + (8 if dout == 8 else 4) * a + dout] = U[:din, kc:kc + dout]
    return S


def _s_h2(U2, G):
    """LSTM2 recurrent stationary [128, 128]:
    S[32g + 4a + r, 32g + 4a + d] = U2[r, kcol2+d]  (r, d < 4; rows 16..31
    of each 32-block unused/zero)."""
    U2 = np.asarray(U2, np.float32)
    kc = KCOL2[G]
    S = np.zeros((128, 128), np.float32)
    for g in range(4):
        for a in range(4):
            r0 = 32 * g + 4 * a
            S[r0:r0 + 4, r0:r0 + 4] = U2[:, kc:kc + 4]
    return S


def _bias_vec(b, G, dout, mod=None):
    kc = (KCOL if dout == 8 else KCOL2)[G]
    mod = dout if mod is None else mod
    v = np.zeros(128, np.float32)
    for p in range(128):
        v[p] = b[kc + (p % mod) % dout]
    return v


def _pack_weights(W0, U0, b0, W1, U1, b1, W2, U2, b2,
                  gamma_h, beta_h, gamma_c, beta_c, gamma3, beta3):
    wd = {}
    wd["s0x"] = np.stack([_s_x0(W0, G) for G in GATES]).astype(BF16)          # [4, 128, 64]
    wd["s0h"] = np.stack([_s_h(U0, G, 8, 8) for G in GATES]).astype(BF16)     # [4, 128, 128]
    W1c = np.asarray(W1, np.float32) + np.asarray(U1, np.float32)
    wd["s1"] = np.stack([_s_h(W1c, G, 8, 8) for G in GATES]).astype(BF16)     # [4, 128, 128]
    # LSTM2 input side: S[32g+8a+r, 32g+4a+d] = W2[r, kc2+d]
    s2x = np.zeros((4, 128, 128), np.float32)
    for Gi, G in enumerate(GATES):
        kc = KCOL2[G]
        for g in range(4):
            for a in range(4):
                s2x[Gi, 32 * g + 8 * a:32 * g + 8 * a + 8,
                    32 * g + 4 * a:32 * g + 4 * a + 4] = \
                    np.asarray(W2, np.float32)[:, kc:kc + 4]
    wd["s2x"] = s2x.astype(BF16)
    wd["s2h"] = np.stack([_s_h2(U2, G) for G in GATES]).astype(BF16)          # [4, 128, 128]
    # W2 gate cols replicated along 128 cols for the beta3 @ W2 bias fold:
    s2b = np.zeros((4, 8, 128), np.float32)
    for Gi, G in enumerate(GATES):
        kc = KCOL2[G]
        for p in range(128):
            s2b[Gi, :, p] = np.asarray(W2, np.float32)[:, kc + (p % 4)]
    wd["s2b"] = s2b
    wd["bias0"] = np.stack([_bias_vec(np.asarray(b0, np.float32), G, 8)
                            for G in GATES], axis=1)            # [128, 4]
    wd["bias1"] = np.stack([_bias_vec(np.asarray(b1, np.float32), G, 8)
                            for G in GATES], axis=1)            # [128, 4]
    wd["bias2"] = np.stack([_bias_vec(np.asarray(b2, np.float32), G, 4)
                            for G in GATES], axis=1)            # [128, 4]
    sel = np.zeros((128, 8), np.float32)
    for p in range(128):
        sel[p, p % 8] = 1.0
    wd["sel"] = sel
    g8 = lambda v: np.asarray(v, np.float32)[np.arange(128) % 8]
    wd["gbv"] = np.stack([g8(gamma_h), g8(beta_h), g8(gamma_c), g8(beta_c),
                          g8(gamma3), g8(beta3)], axis=1)       # [128, 6]
    return wd


def _unpack_out(res_list):
    """res_list: 8 arrays [14, 128, 512] -> out [65536, 14, 4]."""
    res = np.stack([np.asarray(r, np.float32) for r in res_list])
    v = res.reshape(NCORES, ROWS, 4, 32, 512)[:, :, :, :16, :]
    v = v.reshape(NCORES, ROWS, 4, 4, 4, 512)               # [c, r, g, a, d, n]
    v = v.transpose(0, 2, 3, 5, 1, 4)                       # [c, g, a, n, r, d]
    return np.ascontiguousarray(v).reshape(B, ROWS, Do)


# --------------------------------------------------------------------------
# Device kernel
# --------------------------------------------------------------------------

def _build(nc, collectives=True, reps=1, nsplit=2):
    import concourse.mybir as mybir
    import concourse.tile as tile
    from contextlib import ExitStack

    dt = mybir.dt
    AF = mybir.ActivationFunctionType
    ALU = mybir.AluOpType
    AX = mybir.AxisListType
    f32 = dt.float32
    bf = dt.bfloat16

    xt_d = nc.dram_tensor("xt", [T, 128, 1024], bf, kind="ExternalInput")
    s0x_d = nc.dram_tensor("s0x", [4, 128, 64], bf, kind="ExternalInput")
    s0h_d = nc.dram_tensor("s0h", [4, 128, 128], bf, kind="ExternalInput")
    s1_d = nc.dram_tensor("s1", [4, 128, 128], bf, kind="ExternalInput")
    s2x_d = nc.dram_tensor("s2x", [4, 128, 128], bf, kind="ExternalInput")
    s2h_d = nc.dram_tensor("s2h", [4, 128, 128], bf, kind="ExternalInput")
    s2b_d = nc.dram_tensor("s2b", [4, 8, 128], f32, kind="ExternalInput")
    bias0_d = nc.dram_tensor("bias0", [128, 4], f32, kind="ExternalInput")
    bias1_d = nc.dram_tensor("bias1", [128, 4], f32, kind="ExternalInput")
    bias2_d = nc.dram_tensor("bias2", [128, 4], f32, kind="ExternalInput")
    sel_d = nc.dram_tensor("sel", [128, 8], f32, kind="ExternalInput")
    gbv_d = nc.dram_tensor("gbv", [128, 6], f32, kind="ExternalInput")
    out_d = nc.dram_tensor("out", [ROWS, 128, 512], bf, kind="ExternalOutput")
    cc1_in = nc.dram_tensor("cc1_in", [8, 4], f32, kind="Internal")
    cc1_out = nc.dram_tensor("cc1_out", [8, 4], f32, kind="Internal",
                             addr_space="Shared")
    cc2_in = nc.dram_tensor("cc2_in", [8, 2], f32, kind="Internal")
    cc2_out = nc.dram_tensor("cc2_out", [8, 2], f32, kind="Internal",
                             addr_space="Shared")
    RG = [list(range(NCORES))]
    NS = nsplit
    W = 512 // NS              # chain-stream width
    HALVES = [(i * W, (i + 1) * W) for i in range(NS)]

    with tile.TileContext(nc) as tc, ExitStack() as ctx:
        const = ctx.enter_context(tc.tile_pool(name="const", bufs=1))
        xpool = ctx.enter_context(tc.tile_pool(name="xp", bufs=4))
        spool = ctx.enter_context(tc.tile_pool(name="sp", bufs=2))
        vpool = ctx.enter_context(tc.tile_pool(name="vp", bufs=2))
        state = ctx.enter_context(tc.tile_pool(name="st", bufs=1))
        pp = ctx.enter_context(tc.tile_pool(name="pp", bufs=2, space="PSUM"))

        S0X, S0H, S1, S2X, S2H, S2B = {}, {}, {}, {}, {}, {}
        for Gi, G in enumerate(GATES):
            S0X[G] = const.tile([128, 64], bf, tag=f"s0x{G}", name=f"s0x{G}")
            nc.sync.dma_start(S0X[G][:], s0x_d[Gi])
            S0H[G] = const.tile([128, 128], bf, tag=f"s0h{G}", name=f"s0h{G}")
            nc.sync.dma_start(S0H[G][:], s0h_d[Gi])
            S1[G] = const.tile([128, 128], bf, tag=f"s1{G}", name=f"s1{G}")
            nc.sync.dma_start(S1[G][:], s1_d[Gi])
            S2X[G] = const.tile([128, 128], bf, tag=f"s2x{G}", name=f"s2x{G}")
            nc.sync.dma_start(S2X[G][:], s2x_d[Gi])
            S2H[G] = const.tile([128, 128], bf, tag=f"s2h{G}", name=f"s2h{G}")
            nc.sync.dma_start(S2H[G][:], s2h_d[Gi])
            S2B[G] = const.tile([8, 128], f32, tag=f"s2b{G}", name=f"s2b{G}")
            nc.sync.dma_start(S2B[G][:], s2b_d[Gi])
        BIAS0 = const.tile([128, 4], f32, tag="bias0", name="bias0")
        nc.sync.dma_start(BIAS0[:], bias0_d[:])
        BIAS1 = const.tile([128, 4], f32, tag="bias1", name="bias1")
        nc.sync.dma_start(BIAS1[:], bias1_d[:])
        BIAS2 = const.tile([128, 4], f32, tag="bias2", name="bias2")
        nc.sync.dma_start(BIAS2[:], bias2_d[:])
        SEL = const.tile([128, 8], f32, tag="sel", name="sel")
        nc.sync.dma_start(SEL[:], sel_d[:])
        GBV = const.tile([128, 6], f32, tag="gbv", name="gbv")
        nc.sync.dma_start(GBV[:], gbv_d[:])

        H = state.tile([128, 512], bf, tag="H", name="H")
        C = state.tile([128, 512], f32, tag="C", name="C")
        H2 = state.tile([128, 512], bf, tag="H2", name="H2")
        C2 = state.tile([128, 512], bf, tag="C2", name="C2")
        HS = state.tile([128, ROWS * 512], bf, tag="HS", name="HS")
        SUMS3 = state.tile([128, NS * ROWS], f32, tag="SUMS3", name="SUMS3")
        SQS3 = state.tile([128, ROWS], f32, tag="SQS3", name="SQS3")
        GIDX = {G: i for i, G in enumerate(GATES)}

        def gate_act(ps, bias_tile, act, cst, h_full, h_col0, s, sum_col):
            """One chain-stream state update on columns [c0, c1)."""
            c0, c1 = HALVES[s]
            w = c1 - c0
            bcol = lambda G: bias_tile[:, GIDX[G]:GIDX[G] + 1]
            sfx = f"{s}"
            SI = spool.tile([128, w], bf, tag=f"SI{sfx}", name=f"SI{sfx}")
            SF = spool.tile([128, w], bf, tag=f"SF{sfx}", name=f"SF{sfx}")
            SO = spool.tile([128, w], bf, tag=f"SO{sfx}", name=f"SO{sfx}")
            SG = spool.tile([128, w], bf, tag=f"SG{sfx}", name=f"SG{sfx}")
            cbf = cst.dtype == bf
            nc.scalar.activation(SF[:], ps["f"][:, c0:c1], AF.Sigmoid,
                                 bias=bcol("f"))
            if act == "tanh":
                nc.scalar.activation(SG[:], ps["g"][:, c0:c1], AF.Tanh,
                                     bias=bcol("g"))
            else:
                nc.vector.tensor_scalar(SG[:], ps["g"][:, c0:c1], bcol("g"),
                                        0.0, ALU.add, ALU.max)
            nc.scalar.activation(SI[:], ps["i"][:, c0:c1], AF.Sigmoid,
                                 bias=bcol("i"))
            nc.scalar.activation(SO[:], ps["o"][:, c0:c1], AF.Sigmoid,
                                 bias=bcol("o"))
            T2 = vpool.tile([128, w], cst.dtype, tag=f"T2{sfx}",
                            name=f"T2{sfx}")
            nc.vector.tensor_mul(T2[:], cst[:, c0:c1], SF[:])
            M1 = vpool.tile([128, w], bf, tag=f"M1{sfx}", name=f"M1{sfx}")
            nc.vector.tensor_mul(M1[:], SI[:], SG[:])
            nc.vector.tensor_add(cst[:, c0:c1], T2[:], M1[:])
            h_dst = h_full[:, h_col0 + c0:h_col0 + c1]
            if act == "tanh":
                TT = vpool.tile([128, w], bf, tag=f"TT{sfx}", name=f"TT{sfx}")
                nc.scalar.activation(TT[:], cst[:, c0:c1], AF.Tanh)
                nc.vector.tensor_mul(h_dst, SO[:], TT[:])
            elif cbf:
                TR = vpool.tile([128, w], bf, tag=f"TT{sfx}", name=f"TR{sfx}")
                nc.vector.tensor_scalar(TR[:], cst[:, c0:c1], 0.0, None,
                                        ALU.max)
                nc.vector.tensor_mul(h_dst, SO[:], TR[:])
            elif sum_col is None:
                nc.vector.scalar_tensor_tensor(h_dst, cst[:, c0:c1], 0.0,
                                               SO[:], ALU.max, ALU.mult)
            else:
                nc.vector.scalar_tensor_tensor(
                    h_dst, cst[:, c0:c1], 0.0, SO[:], ALU.max, ALU.mult,
                    accum_out=SUMS3[:, sum_col:sum_col + 1])

        def new_ps():
            return {G: pp.tile([128, 512], f32, tag=f"ps{G}",
                               name=f"ps{G}") for G in CHAIN}

        CHAIN = ("f", "g", "i", "o")

        for _rep in range(reps):
            nc.vector.memset(H[:], 0.0)
            nc.vector.memset(C[:], 0.0)

            # ================= LSTM0 =================
            for t in range(T):
                xt_t = xpool.tile([128, 1024], bf, tag="xt", name="xt")
                nc.sync.dma_start(xt_t[:], xt_d[t])
                ps = new_ps()
                for G in CHAIN:
                    nc.tensor.matmul(ps[G][0:64, :], S0X[G][:],
                                     xt_t[:, 0:512], start=True,
                                     stop=(t == 0), skip_group_check=True)
                    nc.tensor.matmul(ps[G][64:128, :], S0X[G][:],
                                     xt_t[:, 512:1024], start=True,
                                     stop=(t == 0), skip_group_check=True)
                for s in range(NS):
                    c0, c1 = HALVES[s]
                    if t > 0:
                        for G in CHAIN:
                            nc.tensor.matmul(ps[G][:, c0:c1], S0H[G][:],
                                             H[:, c0:c1], start=False,
                                             stop=True, skip_group_check=True)
                    gate_act(ps, BIAS0, "tanh", C, H, 0, s, None)

            # ========== BN stats for h, c + AllReduce ==========
            STATS = vpool.tile([128, 4], f32, tag="STATS", name="STATS")
            SQ = vpool.tile([128, 512], f32, tag="SQ", name="SQ")
            nc.vector.reduce_sum(STATS[:, 0:1], H[:], axis=AX.X)
            nc.scalar.activation(SQ[:], H[:], AF.Square,
                                 accum_out=STATS[:, 1:2])
            nc.vector.reduce_sum(STATS[:, 2:3], C[:], axis=AX.X)
            nc.scalar.activation(SQ[:], C[:], AF.Square,
                                 accum_out=STATS[:, 3:4])
            psc = pp.tile([8, 4], f32, tag="psf", name="psc")
            nc.tensor.matmul(psc[:], SEL[:], STATS[:], start=True, stop=True)
            CCS = vpool.tile([8, 4], f32, tag="CCS", name="CCS")
            nc.vector.tensor_copy(CCS[:], psc[:])
            nc.sync.dma_start(cc1_in[:], CCS[:])
            if collectives:
                nc.gpsimd.collective_compute(
                    "AllReduce", ALU.add, replica_groups=RG,
                    ins=[cc1_in.ap()], outs=[cc1_out.ap()])
            else:
                nc.sync.dma_start(cc1_out.ap(), cc1_in.ap())
            MV = vpool.tile([128, 4], f32, tag="MV", name="MV")
            for k in range(16):
                nc.sync.dma_start(MV[8 * k:8 * k + 8, :], cc1_out[:])

            def bn_affine(sum_col, sq_col, gcol, bcol_, count, tagp):
                MEAN = vpool.tile([128, 1], f32, tag=f"mean{tagp}",
                                  name=f"mean{tagp}")
                nc.vector.tensor_scalar(MEAN[:], sum_col, 1.0 / count, None,
                                        ALU.mult)
                VAR = vpool.tile([128, 1], f32, tag=f"var{tagp}",
                                 name=f"var{tagp}")
                nc.vector.tensor_mul(VAR[:], MEAN[:], MEAN[:])
                EX2 = vpool.tile([128, 1], f32, tag=f"ex2{tagp}",
                                 name=f"ex2{tagp}")
                nc.vector.tensor_scalar(EX2[:], sq_col, 1.0 / count, None,
                                        ALU.mult)
                nc.vector.tensor_sub(VAR[:], EX2[:], VAR[:])
                nc.vector.tensor_scalar(VAR[:], VAR[:], 1.0, EPS, ALU.mult,
                                        ALU.add)
                SD = vpool.tile([128, 1], f32, tag=f"sd{tagp}",
                                name=f"sd{tagp}")
                nc.scalar.activation(SD[:], VAR[:], AF.Sqrt)
                RC = vpool.tile([128, 1], f32, tag=f"rc{tagp}",
                                name=f"rc{tagp}")
                nc.vector.reciprocal(RC[:], SD[:])
                A = state.tile([128, 1], f32, tag=f"A{tagp}", name=f"A{tagp}")
                nc.vector.tensor_mul(A[:], gcol, RC[:])
                Bv = state.tile([128, 1], f32, tag=f"B{tagp}",
                                name=f"B{tagp}")
                nc.vector.scalar_tensor_tensor(Bv[:], MEAN[:], -1.0, A[:],
                                               ALU.mult, ALU.mult)
                nc.vector.tensor_add(Bv[:], Bv[:], bcol_)
                return A, Bv

            AH, BH = bn_affine(MV[:, 0:1], MV[:, 1:2], GBV[:, 0:1],
                               GBV[:, 1:2], B, "h")
            AC, BC = bn_affine(MV[:, 2:3], MV[:, 3:4], GBV[:, 2:3],
                               GBV[:, 3:4], B, "c")
            nc.vector.tensor_scalar(H[:], H[:], AH[:], BH[:], ALU.mult,
                                    ALU.add)
            nc.vector.tensor_scalar(C[:], C[:], AC[:], BC[:], ALU.mult,
                                    ALU.add)

            # ================= LSTM1 =================
            for r in range(ROWS):
                ps = new_ps()
                for s in range(NS):
                    c0, c1 = HALVES[s]
                    prev = (H[:, c0:c1] if r == 0
                            else HS[:, 512 * (r - 1) + c0:512 * (r - 1) + c1])
                    for G in CHAIN:
                        nc.tensor.matmul(ps[G][:, c0:c1], S1[G][:], prev,
                                         start=True, stop=True,
                                         skip_group_check=True)
                    gate_act(ps, BIAS1, "relu", C, HS, 512 * r, s,
                             NS * r + s)

            # ========== BN3 stats + AllReduce ==========
            STAT2 = vpool.tile([128, 2], f32, tag="STAT2", name="STAT2")
            for r in range(ROWS):
                nc.scalar.activation(SQ[:], HS[:, 512 * r:512 * r + 512],
                                     AF.Square, accum_out=SQS3[:, r:r + 1])
            nc.vector.reduce_sum(STAT2[:, 0:1], SUMS3[:], axis=AX.X)
            nc.vector.reduce_sum(STAT2[:, 1:2], SQS3[:], axis=AX.X)
            psc2 = pp.tile([8, 2], f32, tag="psf", name="psc2")
            nc.tensor.matmul(psc2[:], SEL[:], STAT2[:], start=True, stop=True)
            CCS2 = vpool.tile([8, 2], f32, tag="CCS2", name="CCS2")
            nc.vector.tensor_copy(CCS2[:], psc2[:])
            nc.sync.dma_start(cc2_in[:], CCS2[:])
            if collectives:
                nc.gpsimd.collective_compute(
                    "AllReduce", ALU.add, replica_groups=RG,
                    ins=[cc2_in.ap()], outs=[cc2_out.ap()])
            else:
                nc.sync.dma_start(cc2_out.ap(), cc2_in.ap())
            MV2 = vpool.tile([128, 2], f32, tag="MV2", name="MV2")
            for k in range(16):
                nc.sync.dma_start(MV2[8 * k:8 * k + 8, :], cc2_out[:])
            A3, B3 = bn_affine(MV2[:, 0:1], MV2[:, 1:2], GBV[:, 4:5],
                               GBV[:, 5:6], B * ROWS, "3")
            BIAS2F = state.tile([128, 4], f32, tag="BIAS2F", name="BIAS2F")
            S2XS = {}
            for Gi2, G in enumerate(GATES):
                S2XS[G] = const.tile([128, 128], bf, tag=f"s2xs{G}",
                                     name=f"s2xs{G}")
                nc.vector.tensor_scalar(S2XS[G][:], S2X[G][:], A3[:], None,
                                        ALU.mult)
                psb = pp.tile([128, 1], f32, tag="pso", name="psb")
                nc.tensor.matmul(psb[:], S2B[G][:], B3[0:8, 0:1],
                                 start=True, stop=True)
                nc.vector.tensor_add(BIAS2F[:, Gi2:Gi2 + 1],
                                     BIAS2[:, Gi2:Gi2 + 1], psb[:])

            # ================= LSTM2 =================
            nc.vector.memset(C2[:], 0.0)
            for rr in range(ROWS):
                boff = 512 * (ROWS - 1 - rr)
                ps = new_ps()
                for s in range(NS):
                    c0, c1 = HALVES[s]
                    for G in CHAIN:
                        nc.tensor.matmul(ps[G][:, c0:c1], S2XS[G][:],
                                         HS[:, boff + c0:boff + c1],
                                         start=True, stop=(rr == 0),
                                         skip_group_check=True)
                    if rr > 0:
                        for G in CHAIN:
                            nc.tensor.matmul(ps[G][:, c0:c1], S2H[G][:],
                                             H2[:, c0:c1], start=False,
                                             stop=True, skip_group_check=True)
                    gate_act(ps, BIAS2F, "relu", C2, H2, 0, s, None)
                nc.sync.dma_start(out_d[rr], H2[:])

    return nc


# --------------------------------------------------------------------------
# Entry point
# --------------------------------------------------------------------------

_CACHE = {}


def _get_compiled():
    if "nc" not in _CACHE:
        import concourse.bacc as bacc
        nc = bacc.Bacc("TRN2", target_bir_lowering=False, debug=False,
                       num_devices=NCORES)
        _build(nc)
        nc.compile()
        _CACHE["nc"] = nc
    return _CACHE["nc"]


def kernel(noise_seed, W0, U0, b0, gamma_h, beta_h, gamma_c, beta_c,
           W1, U1, b1, gamma3, beta3, W2, U2, b2, training=1, **_kw):
    from concourse import bass_utils

    xt = _pack_x(noise_seed)
    wd = _pack_weights(W0, U0, b0, W1, U1, b1, W2, U2, b2,
                       gamma_h, beta_h, gamma_c, beta_c, gamma3, beta3)
    nc = _get_compiled()
    in_maps = []
    for c in range(NCORES):
        m = {"xt": np.ascontiguousarray(xt[c])}
        m.update({k: v for k, v in wd.items()})
        in_maps.append(m)
    trace = bool(int(os.environ.get("KB_TRACE", "0")))
    res = bass_utils.run_bass_kernel_spmd(
        nc, in_maps, core_ids=list(range(NCORES)), trace=trace)
    _CACHE["last_results"] = res
    return _unpack_out([r["out"] for r in res.results])
